# revision 1
# baseline (speedup 1.0000x reference)
"""Multi-head causal self-attention (BS=2, SEQ=2048, DIN=HID=1024, H=16, D=64)
on 8 Trainium2 NeuronCores.

Sharding: (batch x head-group) data+tensor parallel. Core i handles batch
b=i//4 and head group g=i%4 (heads 4g..4g+3). Q/K/V/mix weights are split
column-wise (rows for Wmix) by head group on the host; each core computes its
4 heads' attention plus the partial mix product; the host sums the 4 partial
products per batch and adds bmix.

Device kernel (per core, all matmuls bf16, fp32 accumulation):
  qT/kT = (Wg.T @ x.T) per head pair  -> [128, 2048] (pair-stacked, 64 rows/head)
  v     = x @ Wv + bv                 -> ones-augmented [sk, 65] blocks per head
  scoresT[sk, sq] = kT.T @ qT         (K=64 row-packed head pairs, causal tiles)
  expT  = exp(scoresT / 8)            (ScalarE, diag tiles masked by DVE)
  attU  = [v|1].T @ expT              -> [65, sq]; row 64 = softmax denominator
  attT  = attU[0:64] / attU[64]       (reciprocal + gpsimd partition-broadcast)
  out_partial = attT.T @ Wmix_g       -> [2048, 1024] (bf16)
"""

import sys
import os

for _p in ("/opt/trn_rl_repo", "/root/.axon_site/_ro/trn_rl_repo"):
    if os.path.isdir(_p) and _p not in sys.path:
        sys.path.insert(0, _p)

import numpy as np
import ml_dtypes
from contextlib import ExitStack

import concourse.bass as bass  # noqa: F401  (registers engine methods)
import concourse.mybir as mybir
import concourse.tile as tile
from concourse import bacc
from concourse.bass_utils import run_bass_kernel_spmd

BF16 = mybir.dt.bfloat16
F32 = mybir.dt.float32
nbf = ml_dtypes.bfloat16

BS, SEQ, DIN, HID, H = 2, 2048, 1024, 1024, 16
D = HID // H          # 64
N_CORES = 8
HPG = 4               # heads per group (per core)
GDH = HPG * D         # 256 dh per core
CH = 512              # sq chunk
NCH = SEQ // CH       # 4
NKT = SEQ // 128      # 16 sk tiles
NDT = DIN // 128      # 8 contraction tiles
VW = D + 1            # 65: v + ones column

_CACHE = {}


def build_kernel():
    nc = bacc.Bacc("TRN2", target_bir_lowering=False, debug=False,
                   num_devices=N_CORES)

    xt_d = nc.dram_tensor("xt", [DIN, SEQ], BF16, kind="ExternalInput").ap()
    wq_d = nc.dram_tensor("wq", [DIN, GDH], BF16, kind="ExternalInput").ap()
    wk_d = nc.dram_tensor("wk", [DIN, GDH], BF16, kind="ExternalInput").ap()
    wv_d = nc.dram_tensor("wv", [DIN, GDH], BF16, kind="ExternalInput").ap()
    wm_d = nc.dram_tensor("wm", [GDH, HID], BF16, kind="ExternalInput").ap()
    bqk_d = nc.dram_tensor("bqk", [128, 4], F32, kind="ExternalInput").ap()
    bvr_d = nc.dram_tensor("bvr", [128, GDH], F32, kind="ExternalInput").ap()
    msk_d = nc.dram_tensor("msk", [128, 4 * CH], BF16, kind="ExternalInput").ap()
    out_d = nc.dram_tensor("out", [SEQ, HID], BF16, kind="ExternalOutput").ap()

    with tile.TileContext(nc) as tc:
        with ExitStack() as ctx:
            const = ctx.enter_context(tc.tile_pool(name="const", bufs=1))
            work = ctx.enter_context(tc.tile_pool(name="work", bufs=3))
            expp = ctx.enter_context(tc.tile_pool(name="expp", bufs=4))
            ost = ctx.enter_context(tc.tile_pool(name="ost", bufs=4))

            # ---------------- input loads ----------------
            xt_sb = [const.tile([128, SEQ], BF16, tag=f"xt{k}", name=f"xt{k}")
                     for k in range(NDT)]
            wq_sb = [const.tile([128, GDH], BF16, tag=f"wq{k}", name=f"wq{k}")
                     for k in range(NDT)]
            wk_sb = [const.tile([128, GDH], BF16, tag=f"wk{k}", name=f"wk{k}")
                     for k in range(NDT)]
            wv_sb = [const.tile([128, GDH], BF16, tag=f"wv{k}", name=f"wv{k}")
                     for k in range(NDT)]
            wm_sb = [const.tile([128, HID], BF16, tag=f"wm{p}", name=f"wm{p}")
                     for p in range(2)]
            bqk_sb = const.tile([128, 4], F32, tag="bqk")
            bvr_sb = const.tile([128, GDH], F32, tag="bvr")
            msk_sb = const.tile([128, 4 * CH], BF16, tag="msk")

            for k in range(NDT):
                nc.sync.dma_start(xt_sb[k][:], xt_d[128 * k:128 * (k + 1), :])
                nc.sync.dma_start(wq_sb[k][:], wq_d[128 * k:128 * (k + 1), :])
                nc.sync.dma_start(wk_sb[k][:], wk_d[128 * k:128 * (k + 1), :])
                nc.sync.dma_start(wv_sb[k][:], wv_d[128 * k:128 * (k + 1), :])
            for p in range(2):
                nc.sync.dma_start(wm_sb[p][:], wm_d[128 * p:128 * (p + 1), :])
            nc.sync.dma_start(bqk_sb[:], bqk_d)
            nc.sync.dma_start(bvr_sb[:], bvr_d)
            nc.sync.dma_start(msk_sb[:], msk_d)

            # ---------------- computed persistent tiles ----------------
            qt_sb = [const.tile([128, SEQ], BF16, tag=f"qt{p}", name=f"qt{p}")
                     for p in range(2)]
            kt_sb = [const.tile([128, SEQ], BF16, tag=f"kt{p}", name=f"kt{p}")
                     for p in range(2)]
            # v_aug: per sk-tile, 4 head blocks of [v(64) | ones(1)]
            vaug = const.tile([128, NKT * HPG * VW], BF16, tag="vaug")
            att_sb = [const.tile([128, SEQ], BF16, tag=f"att{p}", name=f"att{p}")
                      for p in range(2)]

            nc.gpsimd.memset(vaug[:], 1.0)

            # ---------------- q/k projections ----------------
            # qT[pair] = Wq[:, 128p:+128].T @ xT ; bias add fused in evac
            with tc.tile_pool(name="projps", bufs=2, space="PSUM") as projps:
                for p in range(2):
                    for iq, (w_sb, bcol) in enumerate(((wq_sb, 0), (wk_sb, 1))):
                        ps = projps.tile([128, SEQ], F32, tag="projps",
                                         name=f"projps{p}{iq}")
                        for k in range(NDT):
                            lhs = w_sb[k][:, 128 * p:128 * (p + 1)]
                            for c in range(NCH):
                                nc.tensor.matmul(
                                    ps[:, CH * c:CH * (c + 1)], lhs,
                                    xt_sb[k][:, CH * c:CH * (c + 1)],
                                    start=(k == 0), stop=(k == NDT - 1))
                        dst = qt_sb[p] if iq == 0 else kt_sb[p]
                        nc.vector.tensor_scalar_add(
                            dst[:], ps[:], bqk_sb[:, 2 * iq + p:2 * iq + p + 1])

            # ---------------- v projection ----------------
            # v[s_tile] = xT[:, s_tile].T @ Wv + bv -> vaug blocks
            with tc.tile_pool(name="vps", bufs=4, space="PSUM") as vps:
                for st in range(NKT):
                    ps = vps.tile([128, GDH], F32, tag="vps", name=f"vps{st}")
                    for k in range(NDT):
                        nc.tensor.matmul(
                            ps[:], xt_sb[k][:, 128 * st:128 * (st + 1)],
                            wv_sb[k][:], start=(k == 0), stop=(k == NDT - 1))
                    # strided write into the 4 [*,65] blocks (ones col preserved)
                    dst = vaug[:, st * HPG * VW:(st + 1) * HPG * VW]
                    dst = dst.rearrange("p (h w) -> p h w", h=HPG)[:, :, 0:D]
                    src = ps.rearrange("p (h d) -> p h d", h=HPG)
                    bsr = bvr_sb.rearrange("p (h d) -> p h d", h=HPG)
                    nc.vector.tensor_add(dst, src, bsr)

            # ---------------- attention ----------------
            with tc.tile_pool(name="sps", bufs=2, space="PSUM") as sps, \
                 tc.tile_pool(name="ups", bufs=4, space="PSUM") as ups:
                for p in range(2):
                    for c in range(NCH):
                        njt = 4 * c + 4          # sk tiles in this chunk
                        pu = [ups.tile([VW, CH], F32, tag="ups",
                                       name=f"ups{p}{c}{hh}") for hh in range(2)]
                        for jg in range(njt // 2):
                            ex = [None, None]
                            for hh in range(2):
                                ps = sps.tile([128, 2 * CH], F32, tag="sps",
                                              name=f"sps{p}{c}{jg}{hh}")
                                for jj in range(2):
                                    j = 2 * jg + jj
                                    nc.tensor.matmul(
                                        ps[:, CH * jj:CH * (jj + 1)],
                                        kt_sb[p][64 * hh:64 * (hh + 1),
                                                 128 * j:128 * (j + 1)],
                                        qt_sb[p][64 * hh:64 * (hh + 1),
                                                 CH * c:CH * (c + 1)],
                                        start=True, stop=True)
                                e = expp.tile([128, 2 * CH], BF16, tag="expT",
                                              name=f"exp{p}{c}{jg}{hh}")
                                nc.scalar.activation(
                                    e[:], ps[:],
                                    mybir.ActivationFunctionType.Exp,
                                    scale=1.0 / (D ** 0.5))
                                # causal mask on diagonal tiles
                                for jj in range(2):
                                    j = 2 * jg + jj
                                    r = j - 4 * c
                                    if r >= 0:
                                        nc.vector.tensor_mul(
                                            e[:, CH * jj:CH * (jj + 1)],
                                            e[:, CH * jj:CH * (jj + 1)],
                                            msk_sb[:, CH * r:CH * (r + 1)])
                                ex[hh] = e
                            for hh in range(2):
                                for jj in range(2):
                                    j = 2 * jg + jj
                                    hg = 2 * p + hh
                                    nc.tensor.matmul(
                                        pu[hh][:],
                                        vaug[:, j * HPG * VW + hg * VW:
                                             j * HPG * VW + (hg + 1) * VW],
                                        ex[hh][:, CH * jj:CH * (jj + 1)],
                                        start=(j == 0), stop=(j == njt - 1))
                        # normalize: att = attU[0:64] / attU[64]
                        for hh in range(2):
                            rec = work.tile([1, CH], F32, tag="rec",
                                            name=f"rec{p}{c}{hh}")
                            nc.vector.reciprocal(rec[:], pu[hh][64:65, :])
                            rb = work.tile([64, CH], F32, tag="recb",
                                           name=f"recb{p}{c}{hh}")
                            nc.gpsimd.partition_broadcast(rb[:], rec[0:1, :])
                            if hh == 0:
                                nc.vector.tensor_mul(
                                    att_sb[p][0:64, CH * c:CH * (c + 1)],
                                    pu[hh][0:64, :], rb[:])
                            else:
                                tmp = work.tile([64, CH], BF16, tag="tmp",
                                                name=f"tmp{p}{c}")
                                nc.vector.tensor_mul(tmp[:], pu[hh][0:64, :],
                                                     rb[:])
                                nc.sync.dma_start(
                                    att_sb[p][64:128, CH * c:CH * (c + 1)],
                                    tmp[:])

            # ---------------- mix ----------------
            with tc.tile_pool(name="mixps", bufs=4, space="PSUM") as mixps:
                for st in range(NKT):
                    pm = [mixps.tile([128, CH], F32, tag="mixps",
                                     name=f"mix{st}{h}") for h in range(2)]
                    for p in range(2):
                        lhs = att_sb[p][:, 128 * st:128 * (st + 1)]
                        for h in range(2):
                            nc.tensor.matmul(
                                pm[h][:], lhs, wm_sb[p][:, CH * h:CH * (h + 1)],
                                start=(p == 0), stop=(p == 1))
                    ot = ost.tile([128, HID], BF16, tag="otile",
                                  name=f"ot{st}")
                    for h in range(2):
                        nc.vector.tensor_copy(ot[:, CH * h:CH * (h + 1)],
                                              pm[h][:])
                    nc.sync.dma_start(out_d[128 * st:128 * (st + 1), :], ot[:])

    nc.compile()
    return nc


def _prep_inputs(x, Wq, bq, Wk, bk, Wv, bv, Wmix):
    """Build the 8 per-core input maps."""
    xt = {b: np.ascontiguousarray(x[b].T).astype(nbf) for b in range(BS)}

    # causal mask tiles: mask[r][p, f] = 1 if f >= 128*r + p else 0
    pp, ff = np.meshgrid(np.arange(128), np.arange(CH), indexing="ij")
    msk = np.concatenate(
        [(ff >= 128 * r + pp).astype(nbf) for r in range(4)], axis=1)
    msk = np.ascontiguousarray(msk)

    in_maps = []
    for i in range(N_CORES):
        b, g = divmod(i, 4)
        sl = slice(GDH * g, GDH * (g + 1))
        bqk = np.stack([bq[GDH * g + 128 * 0: GDH * g + 128 * 1],
                        bq[GDH * g + 128 * 1: GDH * g + 128 * 2],
                        bk[GDH * g + 128 * 0: GDH * g + 128 * 1],
                        bk[GDH * g + 128 * 1: GDH * g + 128 * 2]],
                       axis=1).astype(np.float32)
        in_maps.append({
            "xt": xt[b],
            "wq": np.ascontiguousarray(Wq[:, sl]).astype(nbf),
            "wk": np.ascontiguousarray(Wk[:, sl]).astype(nbf),
            "wv": np.ascontiguousarray(Wv[:, sl]).astype(nbf),
            "wm": np.ascontiguousarray(Wmix[sl, :]).astype(nbf),
            "bqk": np.ascontiguousarray(bqk),
            "bvr": np.ascontiguousarray(
                np.broadcast_to(bv[sl].astype(np.float32), (128, GDH))),
            "msk": msk,
        })
    return in_maps


def kernel(x, Wq, bq, Wk, bk, Wv, bv, Wmix, bmix):
    x = np.asarray(x, np.float32)
    if "nc" not in _CACHE:
        _CACHE["nc"] = build_kernel()
    nc = _CACHE["nc"]

    in_maps = _prep_inputs(np.asarray(x, np.float32),
                           np.asarray(Wq, np.float32), np.asarray(bq, np.float32),
                           np.asarray(Wk, np.float32), np.asarray(bk, np.float32),
                           np.asarray(Wv, np.float32), np.asarray(bv, np.float32),
                           np.asarray(Wmix, np.float32))
    res = run_bass_kernel_spmd(nc, in_maps, core_ids=list(range(N_CORES)))

    out = np.zeros((BS, SEQ, HID), np.float32)
    for i in range(N_CORES):
        out[i // 4] += res.results[i]["out"].astype(np.float32)
    out += np.asarray(bmix, np.float32)
    return out


# revision 13
# speedup vs baseline: 13216.0302x; 13216.0302x over previous
"""Multi-head causal self-attention (BS=2, SEQ=2048, DIN=HID=1024, H=16, D=64)
on 8 Trainium2 NeuronCores.

Sharding: (batch x head-group) data+tensor parallel. Core i handles batch
b=i//4 and head group g=i%4 (heads 4g..4g+3). Q/K/V/mix weights are split
column-wise (rows for Wmix) by head group on the host; each core computes its
4 heads' attention plus the partial mix product; the host sums the 4 partial
products per batch and adds bmix.

Device kernel (per core, all matmuls bf16, fp32 accumulation):
  qT/kT = (Wg.T @ x.T) per head pair  -> [128, 2048] (pair-stacked, 64 rows/head)
  v     = x @ Wv + bv                 -> ones-augmented [sk, 65] blocks per head
  scoresT[sk, sq] = kT.T @ qT         (K=64 row-packed head pairs, causal tiles)
  expT  = exp(scoresT / 8)            (ScalarE, diag triangles masked by DVE)
  attU  = [v|1].T @ expT              -> [65, sq]; row 64 = softmax denominator
  attT  = attU[0:64] / attU[64]       (fast reciprocal + gpsimd partition-bcast)
  out_partial = attT.T @ Wmix_g       -> [2048, 1024] (bf16), mix interleaved
                                         per sq chunk with the attention loop
"""

import sys
import os

for _p in ("/opt/trn_rl_repo", "/root/.axon_site/_ro/trn_rl_repo"):
    if os.path.isdir(_p) and _p not in sys.path:
        sys.path.insert(0, _p)

import numpy as np
import ml_dtypes
from contextlib import ExitStack

import concourse.bass as bass  # noqa: F401  (registers engine methods)
import concourse.mybir as mybir
import concourse.tile as tile
from concourse import bacc
from concourse.bass_utils import run_bass_kernel_spmd

BF16 = mybir.dt.bfloat16
F32 = mybir.dt.float32
nbf = ml_dtypes.bfloat16

BS, SEQ, DIN, HID, H = 2, 2048, 1024, 1024, 16
D = HID // H          # 64
N_CORES = 8
HPG = 4               # heads per group (per core)
GDH = HPG * D         # 256 dh per core
CH = 1024             # sq chunk
NCH = SEQ // CH       # 2
NKT = SEQ // 128      # 16 sk tiles
NDT = DIN // 128      # 8 contraction tiles
VW = D + 1            # 65: v + ones column

_CACHE = {}


def build_kernel():
    nc = bacc.Bacc("TRN2", target_bir_lowering=False, debug=False,
                   num_devices=N_CORES)

    xt_d = nc.dram_tensor("xt", [DIN, SEQ], BF16, kind="ExternalInput").ap()
    wq_d = nc.dram_tensor("wq", [DIN, GDH], BF16, kind="ExternalInput").ap()
    wk_d = nc.dram_tensor("wk", [DIN, GDH], BF16, kind="ExternalInput").ap()
    wv_d = nc.dram_tensor("wv", [DIN, GDH], BF16, kind="ExternalInput").ap()
    wm_d = nc.dram_tensor("wm", [GDH, HID], BF16, kind="ExternalInput").ap()
    bqk_d = nc.dram_tensor("bqk", [128, 4], F32, kind="ExternalInput").ap()
    bvr_d = nc.dram_tensor("bvr", [128, GDH], F32, kind="ExternalInput").ap()
    msk_d = nc.dram_tensor("msk", [128, 4 * 128], BF16, kind="ExternalInput").ap()
    out_d = nc.dram_tensor("out", [SEQ, HID], BF16, kind="ExternalOutput").ap()

    with tile.TileContext(nc) as tc:
        with ExitStack() as ctx:
            const = ctx.enter_context(tc.tile_pool(name="const", bufs=1))
            work = ctx.enter_context(tc.tile_pool(name="work", bufs=3))
            expp = ctx.enter_context(tc.tile_pool(name="expp", bufs=6))
            ost = ctx.enter_context(tc.tile_pool(name="ost", bufs=4))

            # ---------------- input loads ----------------
            xt_sb = [const.tile([128, SEQ], BF16, tag=f"xt{k}", name=f"xt{k}")
                     for k in range(NDT)]
            wq_sb = [const.tile([128, GDH], BF16, tag=f"wq{k}", name=f"wq{k}")
                     for k in range(NDT)]
            wk_sb = [const.tile([128, GDH], BF16, tag=f"wk{k}", name=f"wk{k}")
                     for k in range(NDT)]
            wv_sb = [const.tile([128, GDH], BF16, tag=f"wv{k}", name=f"wv{k}")
                     for k in range(NDT)]
            wm_sb = [const.tile([128, HID], BF16, tag=f"wm{p}", name=f"wm{p}")
                     for p in range(2)]
            bqk_sb = const.tile([128, 4], F32, tag="bqk")
            bvr_sb = const.tile([128, GDH], F32, tag="bvr")
            msk_sb = const.tile([128, 4 * 128], BF16, tag="msk")

            for k in range(NDT):
                nc.sync.dma_start(xt_sb[k][:], xt_d[128 * k:128 * (k + 1), :])
                nc.sync.dma_start(wq_sb[k][:], wq_d[128 * k:128 * (k + 1), :])
                nc.sync.dma_start(wk_sb[k][:], wk_d[128 * k:128 * (k + 1), :])
                nc.sync.dma_start(wv_sb[k][:], wv_d[128 * k:128 * (k + 1), :])
            for p in range(2):
                nc.sync.dma_start(wm_sb[p][:], wm_d[128 * p:128 * (p + 1), :])
            nc.sync.dma_start(bqk_sb[:], bqk_d)
            nc.sync.dma_start(bvr_sb[:], bvr_d)
            nc.sync.dma_start(msk_sb[:], msk_d)

            # ---------------- computed persistent tiles ----------------
            qt_sb = [const.tile([128, SEQ], BF16, tag=f"qt{p}", name=f"qt{p}")
                     for p in range(2)]
            kt_sb = [const.tile([128, SEQ], BF16, tag=f"kt{p}", name=f"kt{p}")
                     for p in range(2)]
            # v_aug: per sk-tile, 4 head blocks of [v(64) | ones(1)]
            vaug = const.tile([128, NKT * HPG * VW], BF16, tag="vaug")
            att_sb = [const.tile([128, SEQ], BF16, tag=f"att{p}", name=f"att{p}")
                      for p in range(2)]

            nc.gpsimd.memset(vaug[:], 1.0)

            # ---------------- q/k projections ----------------
            with tc.tile_pool(name="projps", bufs=2, space="PSUM") as projps:
                for p in range(2):
                    for iq, w_sb in enumerate((wq_sb, wk_sb)):
                        ps = projps.tile([128, SEQ], F32, tag="projps",
                                         name=f"projps{p}{iq}")
                        for k in range(NDT):
                            lhs = w_sb[k][:, 128 * p:128 * (p + 1)]
                            for cc in range(SEQ // 512):
                                nc.tensor.matmul(
                                    ps[:, 512 * cc:512 * (cc + 1)], lhs,
                                    xt_sb[k][:, 512 * cc:512 * (cc + 1)],
                                    start=(k == 0), stop=(k == NDT - 1))
                        dst = qt_sb[p] if iq == 0 else kt_sb[p]
                        nc.vector.tensor_scalar_add(
                            dst[:], ps[:], bqk_sb[:, 2 * iq + p:2 * iq + p + 1])

            # ---------------- v projection ----------------
            with tc.tile_pool(name="vps", bufs=4, space="PSUM") as vps:
                for st in range(NKT):
                    ps = vps.tile([128, GDH], F32, tag="vps", name=f"vps{st}")
                    for k in range(NDT):
                        nc.tensor.matmul(
                            ps[:], xt_sb[k][:, 128 * st:128 * (st + 1)],
                            wv_sb[k][:], start=(k == 0), stop=(k == NDT - 1))
                    # strided write into the 4 [*,65] blocks (ones col kept)
                    dst = vaug[:, st * HPG * VW:(st + 1) * HPG * VW]
                    dst = dst.rearrange("p (h w) -> p h w", h=HPG)[:, :, 0:D]
                    src = ps.rearrange("p (h d) -> p h d", h=HPG)
                    bsr = bvr_sb.rearrange("p (h d) -> p h d", h=HPG)
                    nc.vector.tensor_add(dst, src, bsr)

            # ---------------- attention ----------------
            # CH=1024 sq chunks; per (j, hh) one [128, 1024] scores psum tile,
            # lhsT loaded once per 2 half-MMs downstream. attU accumulates into
            # per-(hh, half) [65, 512] psum banks.
            with tc.tile_pool(name="scp", bufs=2, space="PSUM") as scp, \
                 tc.tile_pool(name="up00", bufs=1, space="PSUM") as up00, \
                 tc.tile_pool(name="up01", bufs=1, space="PSUM") as up01, \
                 tc.tile_pool(name="up10", bufs=1, space="PSUM") as up10, \
                 tc.tile_pool(name="up11", bufs=1, space="PSUM") as up11:
                upool = {(0, 0): up00, (0, 1): up01, (1, 0): up10, (1, 1): up11}
                for c in range(NCH):
                    njt = 8 * c + 8          # sk tiles in this chunk
                    for p in range(2):
                        pu = {}
                        for hh in range(2):
                            for hf in range(2):
                                pu[(hh, hf)] = upool[(hh, hf)].tile(
                                    [VW, 512], F32, tag=f"u{hh}{hf}",
                                    name=f"u{p}{c}{hh}{hf}")
                        for j in range(njt):
                            r = j - 8 * c
                            off = max(0, 128 * r)    # causal col offset
                            for hh in range(2):
                                ps = scp.tile([128, CH], F32, tag="scp",
                                              name=f"scp{p}{c}{j}{hh}")
                                # two N<=512 matmuls off one kT weight load
                                for hf in range(2):
                                    lo = max(off, 512 * hf)
                                    if lo < 512 * (hf + 1):
                                        nc.tensor.matmul(
                                            ps[:, lo:512 * (hf + 1)],
                                            kt_sb[p][64 * hh:64 * (hh + 1),
                                                     128 * j:128 * (j + 1)],
                                            qt_sb[p][64 * hh:64 * (hh + 1),
                                                     CH * c + lo:CH * c + 512 * (hf + 1)],
                                            start=True, stop=True)
                                e = expp.tile([128, CH], BF16, tag="expT",
                                              name=f"exp{p}{c}{j}{hh}")
                                nc.scalar.activation(
                                    e[:, off:], ps[:, off:],
                                    mybir.ActivationFunctionType.Exp,
                                    scale=1.0 / (D ** 0.5))
                                if r >= 0:
                                    nc.vector.tensor_mul(
                                        e[:, off:off + 128],
                                        e[:, off:off + 128],
                                        msk_sb[:, 0:128])
                                hg = 2 * p + hh
                                va = vaug[:, j * HPG * VW + hg * VW:
                                          j * HPG * VW + (hg + 1) * VW]
                                for hf in range(2):
                                    lo = max(off, 512 * hf)
                                    if lo < 512 * (hf + 1):
                                        nc.tensor.matmul(
                                            pu[(hh, hf)][:, lo - 512 * hf:],
                                            va, e[:, lo:512 * (hf + 1)],
                                            start=(j == 0),
                                            stop=(j == min(njt, 8 * c + 4 * (hf + 1)) - 1))
                        # normalize: att = attU[0:64] / attU[64]
                        for hh in range(2):
                            for hf in range(2):
                                u = pu[(hh, hf)]
                                col = CH * c + 512 * hf
                                rec = work.tile([1, 512], F32, tag="rec",
                                                name=f"rec{p}{c}{hh}{hf}")
                                nc.vector.tensor_copy(rec[:], u[64:65, :])
                                rbd = work.tile([64, 512], F32, tag="recbd",
                                                name=f"recbd{p}{c}{hh}{hf}")
                                nc.gpsimd.partition_broadcast(rbd[:], rec[0:1, :])
                                rb = work.tile([64, 512], F32, tag="recb",
                                               name=f"recb{p}{c}{hh}{hf}")
                                nc.vector.reciprocal_approx_fast(rb[:], rbd[:])
                                if hh == 0:
                                    nc.vector.tensor_mul(
                                        att_sb[p][0:64, col:col + 512],
                                        u[0:64, :], rb[:])
                                else:
                                    tmp = work.tile([64, 512], BF16, tag="tmp",
                                                    name=f"tmp{p}{c}{hf}")
                                    nc.vector.tensor_mul(tmp[:], u[0:64, :],
                                                         rb[:])
                                    nc.sync.dma_start(
                                        att_sb[p][64:128, col:col + 512],
                                        tmp[:])

            # ---------------- mix ----------------
            with tc.tile_pool(name="mixps", bufs=4, space="PSUM") as mixps:
                for st in range(NKT):
                    pm = [mixps.tile([128, 512], F32, tag="mixps",
                                     name=f"mix{st}{h}") for h in range(2)]
                    for p in range(2):
                        lhs = att_sb[p][:, 128 * st:128 * (st + 1)]
                        for h in range(2):
                            nc.tensor.matmul(
                                pm[h][:], lhs,
                                wm_sb[p][:, 512 * h:512 * (h + 1)],
                                start=(p == 0), stop=(p == 1))
                    ot = ost.tile([128, HID], BF16, tag="otile",
                                  name=f"ot{st}")
                    for h in range(2):
                        nc.vector.tensor_copy(ot[:, 512 * h:512 * (h + 1)],
                                              pm[h][:])
                    nc.sync.dma_start(out_d[128 * st:128 * (st + 1), :],
                                      ot[:])

    nc.compile()
    return nc


def _prep_inputs(x, Wq, bq, Wk, bk, Wv, bv, Wmix):
    """Build the 8 per-core input maps."""
    xt = {b: np.ascontiguousarray(x[b].T).astype(nbf) for b in range(BS)}

    # causal mask blocks: msk[r][p, f] = 1 if f >= p - ... (block at col 128r:
    # tile col f_global = 128r + f_local, keep f_global >= 128r + p -> f >= p)
    pp, ff = np.meshgrid(np.arange(128), np.arange(128), indexing="ij")
    blk = (ff >= pp).astype(nbf)
    msk = np.ascontiguousarray(np.tile(blk, (1, 4)))

    in_maps = []
    for i in range(N_CORES):
        b, g = divmod(i, 4)
        sl = slice(GDH * g, GDH * (g + 1))
        bqk = np.stack([bq[GDH * g + 128 * 0: GDH * g + 128 * 1],
                        bq[GDH * g + 128 * 1: GDH * g + 128 * 2],
                        bk[GDH * g + 128 * 0: GDH * g + 128 * 1],
                        bk[GDH * g + 128 * 1: GDH * g + 128 * 2]],
                       axis=1).astype(np.float32)
        in_maps.append({
            "xt": xt[b],
            "wq": np.ascontiguousarray(Wq[:, sl]).astype(nbf),
            "wk": np.ascontiguousarray(Wk[:, sl]).astype(nbf),
            "wv": np.ascontiguousarray(Wv[:, sl]).astype(nbf),
            "wm": np.ascontiguousarray(Wmix[sl, :]).astype(nbf),
            "bqk": np.ascontiguousarray(bqk),
            "bvr": np.ascontiguousarray(
                np.broadcast_to(bv[sl].astype(np.float32), (128, GDH))),
            "msk": msk,
        })
    return in_maps


def kernel(x, Wq, bq, Wk, bk, Wv, bv, Wmix, bmix):
    if "nc" not in _CACHE:
        _CACHE["nc"] = build_kernel()
    nc = _CACHE["nc"]

    in_maps = _prep_inputs(np.asarray(x, np.float32),
                           np.asarray(Wq, np.float32), np.asarray(bq, np.float32),
                           np.asarray(Wk, np.float32), np.asarray(bk, np.float32),
                           np.asarray(Wv, np.float32), np.asarray(bv, np.float32),
                           np.asarray(Wmix, np.float32))
    res = run_bass_kernel_spmd(nc, in_maps, core_ids=list(range(N_CORES)))

    out = np.zeros((BS, SEQ, HID), np.float32)
    for i in range(N_CORES):
        out[i // 4] += res.results[i]["out"].astype(np.float32)
    out += np.asarray(bmix, np.float32)
    return out
